# revision 40
# baseline (speedup 1.0000x reference)
"""Trainium2 Bass kernel for MixtureOfAttentionLayer (B=4, S=1024, H=1024,
E=4 attention experts [std-8h, std-12h, linear-8h, local-8h], top-2 gating).

Sharding: 8 cores; core c -> batch b=c//2, head-half p=c%2. Each core computes
its half of every expert's heads for its batch; the host combines core pairs.

Top-2 routing sparsity: the gate selects 2 of 4 experts per token. The host
computes the gate and gathers the selected tokens (queries only - keys/values
still cover the full sequence): experts 0/1/2 as one padded set each, expert 3
(local attention) block-compacted per 256-token block (pad width bw, usually
192) so its banded score/PV structure survives the gather. Per-expert sparse
outputs are scatter-added on the host. Gate weights are applied after the
out-projection as a per-partition scalar folded into the PSUM->SBUF copy.

Schedule (the big wins over the naive per-expert ordering):
- Software pipelining across experts: expert e+1's projection matmul groups
  are interleaved (2 per attention group) into expert e's attention loop, so
  TensorE always has independent work queued while attention waits on
  ScalarE's exp (the attention inner loop is exp-rate-bound).
- Stationary weight ht-tiles stream through a 5-deep rotating pool, letting
  the in-order Sync queue prefetch weights several tiles ahead; wv/wo are
  double-buffered across experts. Output stores are merged per q-tile (bf16)
  and issued from the Scalar HWDGE queue so they never block that prefetch.
- Every SBUF tile is padded to a 128B-multiple per-partition footprint and
  every free-dim stride (npad multiples of 64, wv1 padded 510->512) is kept
  128B-aligned: PE streams ~20% slower on misaligned operands.

The softmax denominator is computed with a bf16 pairwise add-tree on VectorE
(sum of the 8 exp'd k-tiles) followed by a single ones-stationary matmul for
the partition reduction + broadcast. bf16 matmuls, f32 PSUM, transposed score
layout ST=[k,q], no transposes anywhere; bk dropped for softmax experts;
bv folded into a host-side output bias.
"""
import os
import sys
import math
import functools

import numpy as np

for _p in ("/root/.axon_site/_ro/trn_rl_repo", "/opt/trn_rl_repo"):
    if os.path.isdir(_p) and _p not in sys.path:
        sys.path.insert(0, _p)

import types

if "antenv.axon_hooks" not in sys.modules:
    # The image's read-only antenv package lacks axon_hooks; seed it so
    # trn_boot can register the NTFF profile hook (used when trace=True).
    _m = types.ModuleType("antenv.axon_hooks")
    _m._hook = None

    def _set_hook(h, _m=_m):
        _m._hook = h

    def _get_hook(_m=_m):
        return _m._hook

    _m.set_axon_ntff_profile_hook = _set_hook
    _m.get_axon_ntff_profile_hook = _get_hook
    sys.modules["antenv.axon_hooks"] = _m

import ml_dtypes

BF16 = ml_dtypes.bfloat16

P = 128
S = 1024
H = 1024
E = 4
HK = H // P  # 8 H-tiles
NH = [8, 12, 8, 8]
HD = [128, 85, 128, 128]
ATYPE = ["std", "std", "lin", "loc"]
NHC = [4, 6, 4, 4]          # heads per core
PDC = [512, 768, 512, 512]  # padded per-core concat head dim (QT/KT/Wo layout)
PDV = [512, 512, 512, 512]  # per-core V width (e1 zero-padded to 512 cols
                            # so the hk stride stays 128B-aligned)
HDV = [128, 85, 128, 128]   # true per-head V width
WHALF = 32
N_CORES = 8
SCH = [(0, 512), (512, 512)]  # full-S / H chunks


def _chunks(n):
    if n <= 512:
        return [(0, n)]
    c = ((n // 2 + 63) // 64) * 64
    return [(0, c), (c, n - c)]


def _tile_w(Wp):
    """[H, pdc] -> [pdc//P, P, HK*P] per-ht tile-contiguous bf16 layout.

    tiles[ht][p][hk*P + d] = Wp[hk*P + p, ht*P + d]; a single ht tile DMA
    then reads 2 KB contiguous per partition.
    """
    pdc = Wp.shape[1]
    T = Wp.reshape(HK, P, pdc // P, P).transpose(2, 1, 0, 3)
    return np.ascontiguousarray(T.reshape(pdc // P, P, HK * P)).astype(BF16)


# ---------------------------------------------------------------- host prep

def _host_gates(x_b, Wg):
    """x_b [S,H] f32, Wg [H,E] -> gatesT [E,S] f32 (0 for unselected)."""
    logits = x_b @ Wg  # [S, E]
    srt = np.sort(logits, axis=1)
    m1 = srt[:, -1]
    m2 = srt[:, -2]
    den = 1.0 + np.exp(m2 - m1)
    w = np.exp(logits - m1[:, None]) / den[:, None]
    w = np.where(logits >= m2[:, None], w, 0.0)
    return np.ascontiguousarray(w.T.astype(np.float32))  # [E, S]


def _pad_cols(W, hd, heads):
    out = np.zeros((W.shape[0], len(heads) * P), np.float32)
    for i, h in enumerate(heads):
        out[:, i * P : i * P + hd] = W[:, h * hd : (h + 1) * hd]
    return out


def _pad_rows(W, hd, heads):
    out = np.zeros((len(heads) * P, W.shape[1]), np.float32)
    for i, h in enumerate(heads):
        out[i * P : i * P + hd] = W[h * hd : (h + 1) * hd]
    return out


def _pad_vec(v, hd, heads):
    out = np.zeros((len(heads) * P,), np.float32)
    for i, h in enumerate(heads):
        out[i * P : i * P + hd] = v[h * hd : (h + 1) * hd]
    return out


E3KTS = [[kt for kt in range(2 * qc - 1, 2 * qc + 3) if 0 <= kt < HK]
         for qc in range(4)]
E3MOFF = [0]
for _qc in range(4):
    E3MOFF.append(E3MOFF[-1] + len(E3KTS[_qc]))


def _maskc(pos3, bw):
    """Compacted local-attention masks: [P, 14, bw] bf16.

    pos3[qc] = true token positions of block qc's compacted queries
    (padded entries replicate position qc*256, keeping denominators > 0;
    their gate weight is 0 so they never reach the output)."""
    m = np.zeros((P, E3MOFF[-1], bw), np.float32)
    kk = np.arange(P)[:, None]
    for qc in range(4):
        pp = np.full(bw, qc * 256, np.int64)
        pp[: len(pos3[qc])] = pos3[qc]
        for i, kt in enumerate(E3KTS[qc]):
            m[:, E3MOFF[qc] + i, :] = (
                np.abs(kt * P + kk - pp[None, :]) <= WHALF)
    return m.astype(BF16)


def _prep_core(inputs, b, p, gatesT, sel_idx, pos3, npad, bw):
    d = {}
    x_b = inputs["x"][b]
    xT = np.ascontiguousarray(x_b.T).astype(BF16)  # [H, S]
    d["x_t"] = xT
    # gathered query-side inputs for the sparse experts (e3: block-compacted)
    for e in range(3):
        idx = sel_idx[e]
        n = npad[e]
        pidx = np.zeros(n, np.int64)
        pidx[: len(idx)] = idx
        d[f"xg{e}"] = np.ascontiguousarray(xT[:, pidx])
    pidx3 = np.zeros(4 * bw, np.int64)
    for qc in range(4):
        pidx3[qc * bw : qc * bw + len(pos3[qc])] = pos3[qc]
    d["xg3"] = np.ascontiguousarray(xT[:, pidx3])
    d["maskc"] = _maskc(pos3, bw)
    # per-q-tile gate scalars, [P, total_tiles] (0 on padded rows)
    np3 = 4 * bw
    tiles = [npad[0] // P + (npad[0] % P > 0),
             npad[1] // P + (npad[1] % P > 0),
             npad[2] // P + (npad[2] % P > 0), np3 // P]
    gcol = np.zeros((P, sum(tiles)), np.float32)
    off = 0
    for e in range(E):
        if e < 3:
            idx = sel_idx[e]
            gv = np.zeros(npad[e], np.float32)
            gv[: len(idx)] = gatesT[e, idx]
        else:
            gv = np.zeros(np3, np.float32)
            for qc in range(4):
                gv[qc * bw : qc * bw + len(pos3[qc])] = gatesT[3, pos3[qc]]
        for t in range(tiles[e]):
            seg = gv[t * P : (t + 1) * P]
            gcol[: len(seg), off + t] = seg
        off += tiles[e]
    # merged per-partition f32 scalars: bq columns for all experts + gcol
    nbq = sum(PDC[e] // P for e in range(E))
    scal = np.zeros((P, nbq + gcol.shape[1]), np.float32)
    boff = 0
    bk2 = None
    for e in range(E):
        hd, nhc = HD[e], NHC[e]
        heads = list(range(p * nhc, (p + 1) * nhc))
        scale = 1.0 / math.sqrt(hd) if ATYPE[e] in ("std", "loc") else 1.0
        d[f"wq{e}"] = _tile_w(_pad_cols(inputs[f"e{e}_Wq"], hd, heads) * scale)
        bqp = _pad_vec(inputs[f"e{e}_bq"], hd, heads) * scale
        scal[:, boff : boff + PDC[e] // P] = bqp.reshape(-1, P).T
        boff += PDC[e] // P
        wkp = _pad_cols(inputs[f"e{e}_Wk"], hd, heads)
        if e == 2:
            d["wk2"] = np.ascontiguousarray(wkp).astype(BF16)
            bk2 = _pad_vec(inputs["e2_bk"], hd, heads)
        else:
            d[f"wk{e}"] = _tile_w(wkp)
        wvp = np.zeros((H, PDV[e]), np.float32)
        wvp[:, : nhc * hd] = inputs[f"e{e}_Wv"][:, heads[0] * hd : (heads[-1] + 1) * hd]
        d[f"wv{e}"] = np.ascontiguousarray(wvp).astype(BF16)
        d[f"wo{e}"] = np.ascontiguousarray(
            _pad_rows(inputs[f"e{e}_Wo"], hd, heads)).astype(BF16)
    scal[:, nbq:] = gcol
    d["scal"] = scal
    d["bk2"] = np.ascontiguousarray(bk2[None, :]).astype(BF16)
    return d


# ---------------------------------------------------------------- device IR

@functools.lru_cache(maxsize=2)
def _build_nc(n0, n1, n2, bw):
    import concourse.mybir as mybir
    import concourse.tile as tile
    from concourse import bacc

    f32 = mybir.dt.float32
    bf16 = mybir.dt.bfloat16
    Exp = mybir.ActivationFunctionType.Exp
    Copy = mybir.ActivationFunctionType.Copy
    Ident = mybir.ActivationFunctionType.Identity

    NP = [n0, n1, n2, 4 * bw]
    QTILES = [[(q0, min(P, NP[e] - q0)) for q0 in range(0, NP[e], P)]
              for e in range(E)]
    GOFF = [0]
    for e in range(E):
        GOFF.append(GOFF[-1] + len(QTILES[e]))
    CWMAX = max(256, max(cw for e in range(3) for _, cw in _chunks(NP[e])))
    NBQ = sum(PDC[e] // P for e in range(E))
    BQOFF = [sum(PDC[i] // P for i in range(e)) for e in range(E)]
    NSCAL = NBQ + GOFF[-1]

    nc = bacc.Bacc(None, target_bir_lowering=False)

    x_t = nc.declare_dram_parameter("x_t", [H, S], bf16, isOutput=False)
    xg_d = [nc.declare_dram_parameter(f"xg{e}", [H, NP[e]], bf16, isOutput=False)
            for e in range(E)]
    scal_d = nc.declare_dram_parameter("scal", [P, NSCAL], f32, isOutput=False)
    maskc_d = nc.declare_dram_parameter("maskc", [P, E3MOFF[-1], bw], bf16,
                                        isOutput=False)
    bk2_d = nc.declare_dram_parameter("bk2", [1, PDC[2]], bf16, isOutput=False)
    wq_d, wk_d, wv_d, wo_d = [], [], [], []
    for e in range(E):
        wq_d.append(nc.declare_dram_parameter(
            f"wq{e}", [PDC[e] // P, P, HK * P], bf16, isOutput=False))
        if e == 2:
            wk_d.append(nc.declare_dram_parameter(
                "wk2", [H, PDC[2]], bf16, isOutput=False))
        else:
            wk_d.append(nc.declare_dram_parameter(
                f"wk{e}", [PDC[e] // P, P, HK * P], bf16, isOutput=False))
        wv_d.append(nc.declare_dram_parameter(f"wv{e}", [H, PDV[e]], bf16, isOutput=False))
        wo_d.append(nc.declare_dram_parameter(f"wo{e}", [PDC[e], H], bf16, isOutput=False))
    out_d = [nc.declare_dram_parameter(f"out{e}", [NP[e], H], bf16, isOutput=True)
             for e in range(E)]

    def pd_chunks(pdc):
        out, off = [], 0
        while off < pdc:
            w = min(512, pdc - off)
            out.append((off, w))
            off += w
        return out

    with tile.TileContext(nc) as tc:
        with (
            tc.tile_pool(name="singles", bufs=1) as singles,
            tc.tile_pool(name="xgpool", bufs=2) as xgpool,
            tc.tile_pool(name="ospool", bufs=2) as ospool,
            tc.tile_pool(name="wtpool", bufs=5) as wtpool,
            tc.tile_pool(name="wvpool", bufs=2) as wvpool,
            tc.tile_pool(name="wopool", bufs=2) as wopool,
            tc.tile_pool(name="apool", bufs=2) as apool,
            tc.tile_pool(name="otpool", bufs=2) as otpool,
            tc.tile_pool(name="epool", bufs=2) as epool,
            tc.tile_pool(name="tpool", bufs=2) as tpool,
            tc.tile_pool(name="psS", bufs=2, space="PSUM") as psS,
            tc.tile_pool(name="psA", bufs=2, space="PSUM") as psA,
            tc.tile_pool(name="psB", bufs=1, space="PSUM") as psB,
            tc.tile_pool(name="psC", bufs=1, space="PSUM") as psC,
        ):
            # ---- early loads: e3's compacted query gather (xg3) interleaved
            # with its wq ht-tiles so the first projection chain starts as
            # soon as the first half lands; xT (for K/V) follows.
            xT = singles.tile([P, HK, S], bf16)
            x_t_r = x_t.ap().rearrange("(o p) s -> p o s", p=P)

            def wt_dma(wdram, ht):
                wt = wtpool.tile([P, HK, P], bf16, tag="wt", name="wt")
                nc.sync.dma_start(
                    wt[:],
                    wdram.ap()[ht : ht + 1].rearrange(
                        "a p (o d) -> p (a o) d", d=P))
                return wt

            # scal (bq/gcol) goes FIRST: the Q-projection's bias add (and
            # through psA slot reuse, the whole K-projection) blocks on it,
            # and behind the bulk transfers it would land ~15us late.
            scal_sb = singles.tile([P, NSCAL], f32,
                                   padded_shape=[P, (NSCAL + 31) // 32 * 32])
            nc.sync.dma_start(scal_sb[:], scal_d.ap())
            xg3_sb = singles.tile([P, HK, 4 * bw], bf16)
            xg3_r = xg_d[3].ap().rearrange("(o p) n -> p o n", p=P)
            # preload all 4 wq3 tiles (wtpool has 5 slots, so none of these
            # pre-DMAs blocks the in-order Sync queue on slot reuse, and
            # wk3's first tile still has a free slot to issue into)
            wq3_pre = []
            for q in range(4):
                if q < 2:
                    nc.sync.dma_start(xg3_sb[:, 4 * q : 4 * q + 4],
                                      xg3_r[:, 4 * q : 4 * q + 4])
                wq3_pre.append(wt_dma(wq_d[3], q))
            for q in range(4):
                nc.sync.dma_start(xT[:, 2 * q : 2 * q + 2],
                                  x_t_r[:, 2 * q : 2 * q + 2])
            # NSCAL is not a multiple of 32 elems; pad the slot to keep every
            # subsequent SBUF tile 128B-aligned (PE streams ~20% slower on
            # misaligned operands).
            maskc_sb = singles.tile([P, E3MOFF[-1], bw], bf16)
            nc.sync.dma_start(maskc_sb[:], maskc_d.ap())
            bk2_sb = singles.tile([1, PDC[2]], bf16)
            nc.sync.dma_start(bk2_sb[:], bk2_d.ap())

            bq_sb = [scal_sb[:, BQOFF[e] : BQOFF[e] + PDC[e] // P]
                     for e in range(E)]
            gcol_sb = scal_sb[:, NBQ : NBQ + GOFF[-1]]

            def load_w(pool, dram, pdc, tag, trans=False, bufs=None):
                """[H, pdc] -> sbuf [P, HK, pdc]   (or [pdc, H] -> [P, pdc//P, H])"""
                if trans:
                    t = pool.tile([P, pdc // P, H], bf16, tag=tag,
                                  name=f"{tag}_t", bufs=bufs)
                    nc.sync.dma_start(t[:], dram.ap().rearrange("(o p) h -> p o h", p=P))
                else:
                    t = pool.tile([P, HK, pdc], bf16, tag=tag,
                                  name=f"{tag}_w", bufs=bufs)
                    r = dram.ap().rearrange("(o p) d -> p o d", p=P)
                    for half in range(2):
                        nc.sync.dma_start(t[:, 4 * half : 4 * half + 4],
                                          r[:, 4 * half : 4 * half + 4])
                return t

            ones_row = singles.tile([1, P], bf16)
            nc.vector.memset(ones_row[:], 1.0)
            ones_col = singles.tile([P, 1], bf16, padded_shape=[P, 64])
            nc.vector.memset(ones_col[:], 1.0)
            ones_mat = singles.tile([P, P], bf16)
            nc.vector.memset(ones_mat[:], 1.0)

            def proj_T(wdram, pdc, mv, nlen, pre=None):
                """QT/KT-style projection psums: [P(d-cols), chunk] = W.T @ mv.

                Stationary weight ht-tiles stream through wtpool (4-deep), so
                the Sync queue prefetches ahead while TensorE consumes."""
                for ht in range(pdc // P):
                    wt = (pre[ht] if pre and ht < len(pre)
                          else wt_dma(wdram, ht))
                    for (c0, cw) in _chunks(nlen):
                        ps = psA.tile([P, 512], f32, tag="mm", name="proj_ps")
                        for hk in range(HK):
                            nc.tensor.matmul(
                                ps[:, :cw],
                                wt[:, hk, :],
                                mv[:, hk, c0 : c0 + cw],
                                start=(hk == 0),
                                stop=(hk == HK - 1),
                            )
                        yield ps, ht, c0, cw

            def proj_nat(w_sb, pdc, bias_sb=None):
                """V-style natural projection psums: [P(s), chunk] = xT.T @ W."""
                for st in range(HK):
                    for (c0, cw) in pd_chunks(pdc):
                        ps = psA.tile([P, 512], f32, tag="mm", name="projn_ps")
                        for hk in range(HK):
                            nc.tensor.matmul(
                                ps[:, :cw],
                                xT[:, hk, st * P : (st + 1) * P],
                                w_sb[:, hk, c0 : c0 + cw],
                                start=(hk == 0),
                                stop=(hk == HK - 1 and bias_sb is None),
                            )
                        if bias_sb is not None:
                            nc.tensor.matmul(
                                ps[:, :cw],
                                ones_row[:, :P],
                                bias_sb[:, c0 : c0 + cw],
                                start=False, stop=True,
                            )
                        yield ps, st, c0, cw

            def elu_p1(ps, dst_ap, cw, bias=None):
                """dst = elu(ps + bias)+1 = exp(min(.,0)) + max(.,0), bf16."""
                tmin = tpool.tile([P, 512], bf16, tag="tmin", name="tmin", bufs=1)
                texp = tpool.tile([P, 512], bf16, tag="texp", name="texp", bufs=1)
                tmax = tpool.tile([P, 512], bf16, tag="tmax", name="tmax", bufs=1)
                if bias is None:
                    nc.vector.tensor_scalar_min(tmin[:, :cw], ps[:, :cw], 0.0)
                    nc.vector.tensor_scalar_max(tmax[:, :cw], ps[:, :cw], 0.0)
                else:
                    nc.vector.tensor_scalar(
                        tmin[:, :cw], ps[:, :cw], bias, 0.0,
                        mybir.AluOpType.add, mybir.AluOpType.min)
                    nc.vector.tensor_scalar(
                        tmax[:, :cw], ps[:, :cw], bias, 0.0,
                        mybir.AluOpType.add, mybir.AluOpType.max)
                nc.scalar.activation(texp[:, :cw], tmin[:, :cw], Exp)
                nc.vector.tensor_add(dst_ap, texp[:, :cw], tmax[:, :cw])

            def tree_sum(est, k0, nk, cw):
                """Sum est slots [P, k0:k0+nk, :cw] (bf16) -> [P, cw] AP.
                Wide 3D-AP adds keep the DVE instruction count at 2-3."""
                tr = tpool.tile([P, 4, CWMAX], bf16, tag="tr", name="tr", bufs=1)
                if nk == 8:
                    nc.vector.tensor_add(tr[:, 0:4, :cw], est[:, 0:4, :cw],
                                         est[:, 4:8, :cw])
                    nc.vector.tensor_add(tr[:, 0:2, :cw], tr[:, 0:2, :cw],
                                         tr[:, 2:4, :cw])
                elif nk == 4:
                    nc.vector.tensor_add(tr[:, 0:2, :cw],
                                         est[:, k0 : k0 + 2, :cw],
                                         est[:, k0 + 2 : k0 + 4, :cw])
                else:  # nk == 3
                    nc.vector.tensor_add(tr[:, 0, :cw], est[:, k0, :cw],
                                         est[:, k0 + 1, :cw])
                    nc.vector.tensor_add(tr[:, 0, :cw], tr[:, 0, :cw],
                                         est[:, k0 + 2, :cw])
                    return tr[:, 0, :cw]
                nc.vector.tensor_add(tr[:, 0, :cw], tr[:, 0, :cw], tr[:, 1, :cw])
                return tr[:, 0, :cw]

            def out_proj(e, ot, wo_sb, ti):
                q0, qp = QTILES[e][ti]
                gs = gcol_sb[:qp, GOFF[e] + ti : GOFF[e] + ti + 1]
                o_st = ospool.tile([P, 2, 512], bf16, tag="osb", name="o_st")
                for ci, (c0, cw) in enumerate(SCH):
                    ps = psA.tile([P, 512], f32, tag="mm", name="out_ps")
                    npt = PDC[e] // P
                    for pt in range(npt):
                        nc.tensor.matmul(
                            ps[:qp, :cw],
                            ot[:, pt, q0 : q0 + qp],
                            wo_sb[:, pt, c0 : c0 + cw],
                            start=(pt == 0), stop=(pt == npt - 1),
                        )
                    if ci == 0:
                        nc.scalar.activation(o_st[:qp, 0, :cw], ps[:qp, :cw],
                                             Copy, scale=gs)
                    else:
                        nc.vector.tensor_scalar_mul(o_st[:qp, 1, :cw],
                                                    ps[:qp, :cw], gs)
                # one merged [qp, 1024] bf16 store per q-tile, issued from the
                # Scalar HWDGE queue so weight prefetch on Sync never waits
                nc.scalar.dma_start(out_d[e].ap()[q0 : q0 + qp, :],
                                    o_st[:qp].rearrange("p a b -> p (a b)"))

            # ================= per-expert compute =================
            # Software-pipelined: expert e+1's projection matmul groups are
            # interleaved into expert e's attention loop, so TensorE has
            # independent work queued whenever attention stalls on ScalarE's
            # exp (the attention inner loop is exp-rate-bound).
            ctx = {}

            def proj_gen(e):
                """Emit expert e's projections / weight loads in steps."""
                pdc = PDC[e]
                pdv = PDV[e]
                nq = NP[e]
                if e != 3:
                    xg = xgpool.tile([P, HK, nq], bf16, tag="xg", name=f"xg{e}")
                    r = xg_d[e].ap().rearrange("(o p) n -> p o n", p=P)
                    for half in range(2):
                        nc.sync.dma_start(xg[:, 4 * half : 4 * half + 4],
                                          r[:, 4 * half : 4 * half + 4])
                else:
                    xg = xg3_sb

                if ATYPE[e] in ("std", "loc"):
                    QT = apool.tile([P, pdc // P, nq], bf16, tag="qt", name="QT")
                    for ps, ht, c0, cw in proj_T(wq_d[e], pdc, xg, nq,
                                                 pre=wq3_pre if e == 3 else None):
                        nc.vector.tensor_scalar_add(
                            QT[:, ht, c0 : c0 + cw], ps[:, :cw],
                            bq_sb[e][:, ht : ht + 1])
                        yield
                    KT = apool.tile([P, pdc // P, S], bf16, tag="kt", name="KT")
                    for ps, ht, c0, cw in proj_T(wk_d[e], pdc, xT, S):
                        nc.vector.tensor_copy(KT[:, ht, c0 : c0 + cw], ps[:, :cw])
                        yield
                else:  # linear: q' = elu(QT+bq)+1 ; k' natural = elu(K+bk)+1
                    QT = apool.tile([P, pdc // P, nq], bf16, tag="qt", name="QTl")
                    for ps, ht, c0, cw in proj_T(wq_d[e], pdc, xg, nq):
                        elu_p1(ps, QT[:, ht, c0 : c0 + cw], cw,
                               bias=bq_sb[e][:, ht : ht + 1])
                        yield
                    wk2_sb = load_w(wvpool, wk_d[2], PDC[2], "wk2", bufs=1)
                    KT = apool.tile([P, HK, pdc], bf16, tag="kt", name="Kn")
                    for ps, st, c0, cw in proj_nat(wk2_sb, pdc, bias_sb=bk2_sb):
                        elu_p1(ps, KT[:, st, c0 : c0 + cw], cw)
                        yield
                wv = load_w(wvpool, wv_d[e], pdv, "wv")
                V = apool.tile([P, HK, pdv], bf16, tag="v", name="V")
                for ps, st, c0, cw in proj_nat(wv, pdv):
                    nc.vector.tensor_copy(V[:, st, c0 : c0 + cw], ps[:, :cw])
                    yield
                ot = otpool.tile([P, pdc // P, nq], bf16, tag="ot", name=f"ot{e}")
                if HDV[e] < P:
                    # packed V: OT pad rows are never written; zero whole tile
                    nc.vector.memset(ot[:], 0.0)
                wo_sb = load_w(wopool, wo_d[e], pdc, "wo", trans=True)
                ctx[e] = (QT, KT, V, ot, wo_sb)

            def attn_gen(e):
                """Emit expert e's attention; yields after each matmul group."""
                pdc = PDC[e]
                hdv = HDV[e]
                nhc = NHC[e]
                nq = NP[e]
                QT, KT, V, ot, wo_sb = ctx[e]
                emitted = set()

                def emit_covered(upto):
                    for ti, (q0, qp) in enumerate(QTILES[e]):
                        if ti not in emitted and q0 + qp <= upto:
                            out_proj(e, ot, wo_sb, ti)
                            emitted.add(ti)

                if ATYPE[e] == "std":
                    for (c0, cw) in _chunks(nq):
                        for h in range(nhc):
                            est = epool.tile([P, HK, CWMAX], bf16, tag="est",
                                             name="est")
                            for kp in range(HK // 2):
                                st_ps = psS.tile([P, 2, 512], f32, tag="sc",
                                                 name="st_ps")
                                for sl in range(2):
                                    kt = 2 * kp + sl
                                    nc.tensor.matmul(
                                        st_ps[:, sl, :cw],
                                        KT[:, h, kt * P : (kt + 1) * P],
                                        QT[:, h, c0 : c0 + cw],
                                        start=True, stop=True,
                                    )
                                nc.scalar.activation(
                                    est[:, 2 * kp : 2 * kp + 2, :cw],
                                    st_ps[:, :, :cw], Exp)
                            o_ps = psB.tile([P, 512], f32, tag="ot", name="o_ps")
                            for kt in range(HK):
                                nc.tensor.matmul(
                                    o_ps[:hdv, :cw],
                                    V[:, kt, h * hdv : (h + 1) * hdv],
                                    est[:, kt, :cw],
                                    start=(kt == 0), stop=(kt == HK - 1),
                                )
                            es = tree_sum(est, 0, HK, cw)
                            den = psC.tile([P, 512], f32, tag="den", name="den")
                            nc.tensor.matmul(den[:hdv, :cw], ones_mat[:, :hdv],
                                             es, start=True, stop=True)
                            rcp = tpool.tile([P, 512], f32, tag="rcp", name="rcp")
                            nc.vector.reciprocal_approx_fast(
                                out=rcp[:hdv, :cw], in_=den[:hdv, :cw])
                            nc.vector.tensor_mul(
                                ot[:hdv, h, c0 : c0 + cw],
                                o_ps[:hdv, :cw], rcp[:hdv, :cw])
                            yield
                        emit_covered(c0 + cw)

                elif ATYPE[e] == "loc":
                    for qc in range(4):
                        kts = E3KTS[qc]
                        nm = len(kts)
                        for h in range(nhc):
                            est = epool.tile([P, 4, bw], bf16, tag="estl",
                                             name="estl")
                            for i0 in range(0, nm, 2):
                                grp = kts[i0 : i0 + 2]
                                st_ps = psS.tile([P, 2, 512], f32, tag="sc",
                                                 name="stl_ps")
                                for sl, kt in enumerate(grp):
                                    nc.tensor.matmul(
                                        st_ps[:, sl, :bw],
                                        KT[:, h, kt * P : (kt + 1) * P],
                                        QT[:, h, qc * bw : (qc + 1) * bw],
                                        start=True, stop=True,
                                    )
                                if len(grp) == 2:
                                    nc.scalar.activation(
                                        est[:, i0 : i0 + 2, :],
                                        st_ps[:, :, :bw], Exp)
                                else:
                                    nc.scalar.activation(
                                        est[:, i0, :], st_ps[:, 0, :bw], Exp)
                            nc.vector.tensor_mul(
                                est[:, 0:nm, :],
                                est[:, 0:nm, :],
                                maskc_sb[:, E3MOFF[qc] : E3MOFF[qc] + nm, :])
                            o_ps = psB.tile([P, 512], f32, tag="ot", name="ol_ps")
                            for i, kt in enumerate(kts):
                                nc.tensor.matmul(
                                    o_ps[:, :bw],
                                    V[:, kt, h * P : (h + 1) * P],
                                    est[:, i, :],
                                    start=(i == 0), stop=(i == nm - 1),
                                )
                            es = tree_sum(est, 0, nm, bw)
                            den = psC.tile([P, 512], f32, tag="den", name="denl")
                            nc.tensor.matmul(den[:, :bw], ones_mat[:],
                                             es, start=True, stop=True)
                            rcp = tpool.tile([P, 512], f32, tag="rcp", name="rcpl")
                            nc.vector.reciprocal_approx_fast(
                                out=rcp[:, :bw], in_=den[:, :bw])
                            nc.vector.tensor_mul(
                                ot[:, h, qc * bw : (qc + 1) * bw],
                                o_ps[:, :bw], rcp[:, :bw])
                            yield
                        emit_covered((qc + 1) * bw)

                else:  # linear
                    nhc_ = nhc
                    kv_sb = tpool.tile([P, 4, P], bf16, tag="kv_sb", name="kv_sb", bufs=1)
                    ks_bc = tpool.tile([P, 4, P], bf16, tag="ks_bc", name="ks_bc", bufs=1)
                    # ksum row [1, pdv] = sum_s k' for all heads in one sweep
                    pdv = PDV[e]
                    ksr_ps = psC.tile([P, 512], f32, tag="den", name="ksr_ps")
                    for st in range(HK):
                        nc.tensor.matmul(
                            ksr_ps[:1, :pdv],
                            ones_col[:, :],
                            KT[:, st, :pdv],
                            start=(st == 0), stop=(st == HK - 1),
                        )
                    ksr_sb = tpool.tile([1, 512], bf16, tag="ksr", name="ksr_sb", bufs=1)
                    nc.scalar.activation(ksr_sb[:, :pdv], ksr_ps[:1, :pdv], Copy)
                    yield
                    for h in range(nhc_):
                        kv_ps = psB.tile([P, 512], f32, tag="ot", name="kv_ps")
                        for st in range(HK):
                            nc.tensor.matmul(
                                kv_ps[:, :P],
                                KT[:, st, h * P : (h + 1) * P],
                                V[:, st, h * P : (h + 1) * P],
                                start=(st == 0), stop=(st == HK - 1),
                            )
                        nc.scalar.activation(kv_sb[:, h, :], kv_ps[:, :P], Copy)
                        # broadcast ksum row across partitions -> [P, P] lhsT
                        kb_ps = psC.tile([P, 512], f32, tag="den", name="kb_ps")
                        nc.tensor.matmul(
                            kb_ps[:, :P],
                            ksr_sb[:, h * P : (h + 1) * P],
                            ones_row[:, :P],
                            start=True, stop=True,
                        )
                        nc.scalar.activation(ks_bc[:, h, :], kb_ps[:, :P], Copy)
                        yield
                    for (c0, cw) in _chunks(nq):
                        for h in range(nhc_):
                            num_ps = psA.tile([P, 512], f32, tag="mm",
                                              name="num_ps")
                            nc.tensor.matmul(
                                num_ps[:, :cw],
                                kv_sb[:, h, :],
                                QT[:, h, c0 : c0 + cw],
                                start=True, stop=True,
                            )
                            den = psC.tile([P, 512], f32, tag="den", name="den2")
                            nc.tensor.matmul(
                                den[:, :cw],
                                ks_bc[:, h, :],
                                QT[:, h, c0 : c0 + cw],
                                start=True, stop=True,
                            )
                            rcp = tpool.tile([P, 512], f32, tag="rcp", name="rcp2")
                            nc.vector.reciprocal_approx_fast(
                                out=rcp[:, :cw], in_=den[:, :cw])
                            nc.vector.tensor_mul(
                                ot[:, h, c0 : c0 + cw],
                                num_ps[:, :cw], rcp[:, :cw])
                            yield
                        emit_covered(c0 + cw)

            ORDER = (3, 0, 1, 2)
            for _ in proj_gen(3):
                pass
            for idx, e in enumerate(ORDER):
                nxt = ORDER[idx + 1] if idx + 1 < len(ORDER) else None
                pg = proj_gen(nxt) if nxt is not None else None
                for gi, _ in enumerate(attn_gen(e)):
                    if pg is not None:
                        for _ in range(2):
                            if next(pg, "done") == "done":
                                pg = None
                                break
                if pg is not None:
                    for _ in pg:
                        pass

    nc.finalize()
    return nc


# ---------------------------------------------------------------- entry

def kernel(**inputs) -> np.ndarray:
    from concourse.bass_utils import run_bass_kernel_spmd

    inputs = {k: np.asarray(v, np.float32) if np.asarray(v).dtype.kind == "f"
              else np.asarray(v) for k, v in inputs.items()}
    gatesT = [_host_gates(inputs["x"][b], inputs["Wg"]) for b in range(4)]
    bo_eff_all = np.stack([
        inputs[f"e{e}_bv"] @ inputs[f"e{e}_Wo"] + inputs[f"e{e}_bo"]
        for e in range(E)
    ])
    # top-2 selection indices per (batch, expert); shared padding across batches
    sel = [[np.nonzero(gatesT[b][e] > 0)[0] for e in range(3)] for b in range(4)]
    # 64-granular padding keeps every per-head free-dim stride a multiple of
    # 128 bytes (SBUF-aligned PE streaming)
    npad = [max(P, -(-max(len(sel[b][e]) for b in range(4)) // 64) * 64)
            for e in range(3)]
    # e3 (local attention): block-compacted queries, one pad width per
    # 256-token block across all batches
    pos3 = [[np.nonzero(gatesT[b][3][qc * 256 : (qc + 1) * 256] > 0)[0]
             + qc * 256 for qc in range(4)] for b in range(4)]
    bw = max(64, -(-max(len(pos3[b][qc]) for b in range(4) for qc in range(4))
                   // 64) * 64)
    in_maps = [
        _prep_core(inputs, c // 2, c % 2, gatesT[c // 2],
                   sel[c // 2], pos3[c // 2], npad, bw)
        for c in range(N_CORES)
    ]
    nc = _build_nc(npad[0], npad[1], npad[2], bw)
    trace = bool(int(os.environ.get("KERNEL_TRACE", "0")))
    if trace:
        import jax

        jax.devices()  # force axon platform registration
        try:
            from antenv.axon_hooks import (
                get_axon_ntff_profile_hook,
                set_axon_ntff_profile_hook,
            )

            if get_axon_ntff_profile_hook() is None:
                from trn_agent_boot.trn_boot import _ntff_profile_via_ctypes

                set_axon_ntff_profile_hook(
                    _ntff_profile_via_ctypes("/opt/axon/libaxon_pjrt.so"))
        except Exception as exc:  # tracing is best-effort
            print(f"NTFF hook setup failed: {exc}")
    res = run_bass_kernel_spmd(nc, in_maps, list(range(N_CORES)), trace=trace)
    if trace and res.exec_time_ns is not None:
        print(f"HW exec time: {res.exec_time_ns} ns")
    out = np.empty((4, S, H), np.float32)
    for b in range(4):
        acc = gatesT[b].T @ bo_eff_all  # gated output-bias term, host-side
        for p in range(2):
            r = res.results[2 * b + p]
            o3 = np.asarray(r["out3"], np.float32)
            for qc in range(4):
                idx = pos3[b][qc]
                acc[idx] += o3[qc * bw : qc * bw + len(idx)]
            for e in range(3):
                idx = sel[b][e]
                acc[idx] += np.asarray(r[f"out{e}"][: len(idx)], np.float32)
        out[b] = acc
    return out



# revision 41
# speedup vs baseline: 1.0097x; 1.0097x over previous
"""Trainium2 Bass kernel for MixtureOfAttentionLayer (B=4, S=1024, H=1024,
E=4 attention experts [std-8h, std-12h, linear-8h, local-8h], top-2 gating).

Sharding: 8 cores; core c -> batch b=c//2, head-half p=c%2. Each core computes
its half of every expert's heads for its batch; the host combines core pairs.

Top-2 routing sparsity: the gate selects 2 of 4 experts per token. The host
computes the gate and gathers the selected tokens (queries only - keys/values
still cover the full sequence): experts 0/1/2 as one padded set each, expert 3
(local attention) block-compacted per 256-token block (pad width bw, usually
192) so its banded score/PV structure survives the gather. Per-expert sparse
outputs are scatter-added on the host. Gate weights are applied after the
out-projection as a per-partition scalar folded into the PSUM->SBUF copy.

Schedule (the big wins over the naive per-expert ordering):
- Software pipelining across experts: expert e+1's projection matmul groups
  are interleaved (2 per attention group) into expert e's attention loop, so
  TensorE always has independent work queued while attention waits on
  ScalarE's exp (the attention inner loop is exp-rate-bound).
- Stationary weight ht-tiles stream through a 5-deep rotating pool, letting
  the in-order Sync queue prefetch weights several tiles ahead; wv/wo are
  double-buffered across experts. Output stores are merged per q-tile (bf16)
  and issued from the Scalar HWDGE queue so they never block that prefetch.
- Every SBUF tile is padded to a 128B-multiple per-partition footprint and
  every free-dim stride (npad multiples of 64, wv1 padded 510->512) is kept
  128B-aligned: PE streams ~20% slower on misaligned operands.

The softmax denominator is computed with a bf16 pairwise add-tree on VectorE
(sum of the 8 exp'd k-tiles) followed by a single ones-stationary matmul for
the partition reduction + broadcast. bf16 matmuls, f32 PSUM, transposed score
layout ST=[k,q], no transposes anywhere; bk dropped for softmax experts;
bv folded into a host-side output bias.
"""
import os
import sys
import math
import functools

import numpy as np

for _p in ("/root/.axon_site/_ro/trn_rl_repo", "/opt/trn_rl_repo"):
    if os.path.isdir(_p) and _p not in sys.path:
        sys.path.insert(0, _p)

import types

if "antenv.axon_hooks" not in sys.modules:
    # The image's read-only antenv package lacks axon_hooks; seed it so
    # trn_boot can register the NTFF profile hook (used when trace=True).
    _m = types.ModuleType("antenv.axon_hooks")
    _m._hook = None

    def _set_hook(h, _m=_m):
        _m._hook = h

    def _get_hook(_m=_m):
        return _m._hook

    _m.set_axon_ntff_profile_hook = _set_hook
    _m.get_axon_ntff_profile_hook = _get_hook
    sys.modules["antenv.axon_hooks"] = _m

import ml_dtypes

BF16 = ml_dtypes.bfloat16

P = 128
S = 1024
H = 1024
E = 4
HK = H // P  # 8 H-tiles
NH = [8, 12, 8, 8]
HD = [128, 85, 128, 128]
ATYPE = ["std", "std", "lin", "loc"]
NHC = [4, 6, 4, 4]          # heads per core
PDC = [512, 768, 512, 512]  # padded per-core concat head dim (QT/KT/Wo layout)
PDV = [512, 512, 512, 512]  # per-core V width (e1 zero-padded to 512 cols
                            # so the hk stride stays 128B-aligned)
HDV = [128, 85, 128, 128]   # true per-head V width
WHALF = 32
N_CORES = 8
SCH = [(0, 512), (512, 512)]  # full-S / H chunks


def _chunks(n):
    if n <= 512:
        return [(0, n)]
    c = ((n // 2 + 63) // 64) * 64
    return [(0, c), (c, n - c)]


def _tile_w(Wp):
    """[H, pdc] -> [pdc//P, P, HK*P] per-ht tile-contiguous bf16 layout.

    tiles[ht][p][hk*P + d] = Wp[hk*P + p, ht*P + d]; a single ht tile DMA
    then reads 2 KB contiguous per partition.
    """
    pdc = Wp.shape[1]
    T = Wp.reshape(HK, P, pdc // P, P).transpose(2, 1, 0, 3)
    return np.ascontiguousarray(T.reshape(pdc // P, P, HK * P)).astype(BF16)


# ---------------------------------------------------------------- host prep

def _host_gates(x_b, Wg):
    """x_b [S,H] f32, Wg [H,E] -> gatesT [E,S] f32 (0 for unselected)."""
    logits = x_b @ Wg  # [S, E]
    srt = np.sort(logits, axis=1)
    m1 = srt[:, -1]
    m2 = srt[:, -2]
    den = 1.0 + np.exp(m2 - m1)
    w = np.exp(logits - m1[:, None]) / den[:, None]
    w = np.where(logits >= m2[:, None], w, 0.0)
    return np.ascontiguousarray(w.T.astype(np.float32))  # [E, S]


def _pad_cols(W, hd, heads):
    out = np.zeros((W.shape[0], len(heads) * P), np.float32)
    for i, h in enumerate(heads):
        out[:, i * P : i * P + hd] = W[:, h * hd : (h + 1) * hd]
    return out


def _pad_rows(W, hd, heads):
    out = np.zeros((len(heads) * P, W.shape[1]), np.float32)
    for i, h in enumerate(heads):
        out[i * P : i * P + hd] = W[h * hd : (h + 1) * hd]
    return out


def _pad_vec(v, hd, heads):
    out = np.zeros((len(heads) * P,), np.float32)
    for i, h in enumerate(heads):
        out[i * P : i * P + hd] = v[h * hd : (h + 1) * hd]
    return out


E3KTS = [[kt for kt in range(2 * qc - 1, 2 * qc + 3) if 0 <= kt < HK]
         for qc in range(4)]
E3MOFF = [0]
for _qc in range(4):
    E3MOFF.append(E3MOFF[-1] + len(E3KTS[_qc]))


def _maskc(pos3, bw):
    """Compacted local-attention masks: [P, 14, bw] bf16.

    pos3[qc] = true token positions of block qc's compacted queries
    (padded entries replicate position qc*256, keeping denominators > 0;
    their gate weight is 0 so they never reach the output)."""
    m = np.zeros((P, E3MOFF[-1], bw), np.float32)
    kk = np.arange(P)[:, None]
    for qc in range(4):
        pp = np.full(bw, qc * 256, np.int64)
        pp[: len(pos3[qc])] = pos3[qc]
        for i, kt in enumerate(E3KTS[qc]):
            m[:, E3MOFF[qc] + i, :] = (
                np.abs(kt * P + kk - pp[None, :]) <= WHALF)
    return m.astype(BF16)


def _prep_core(inputs, b, p, gatesT, sel_idx, pos3, npad, bw):
    d = {}
    x_b = inputs["x"][b]
    xT = np.ascontiguousarray(x_b.T).astype(BF16)  # [H, S]
    d["x_t"] = xT
    # gathered query-side inputs for the sparse experts (e3: block-compacted)
    for e in range(3):
        idx = sel_idx[e]
        n = npad[e]
        pidx = np.zeros(n, np.int64)
        pidx[: len(idx)] = idx
        d[f"xg{e}"] = np.ascontiguousarray(xT[:, pidx])
    pidx3 = np.zeros(4 * bw, np.int64)
    for qc in range(4):
        pidx3[qc * bw : qc * bw + len(pos3[qc])] = pos3[qc]
    d["xg3"] = np.ascontiguousarray(xT[:, pidx3])
    d["maskc"] = _maskc(pos3, bw)
    # per-q-tile gate scalars, [P, total_tiles] (0 on padded rows)
    np3 = 4 * bw
    tiles = [npad[0] // P + (npad[0] % P > 0),
             npad[1] // P + (npad[1] % P > 0),
             npad[2] // P + (npad[2] % P > 0), np3 // P]
    gcol = np.zeros((P, sum(tiles)), np.float32)
    off = 0
    for e in range(E):
        if e < 3:
            idx = sel_idx[e]
            gv = np.zeros(npad[e], np.float32)
            gv[: len(idx)] = gatesT[e, idx]
        else:
            gv = np.zeros(np3, np.float32)
            for qc in range(4):
                gv[qc * bw : qc * bw + len(pos3[qc])] = gatesT[3, pos3[qc]]
        for t in range(tiles[e]):
            seg = gv[t * P : (t + 1) * P]
            gcol[: len(seg), off + t] = seg
        off += tiles[e]
    # merged per-partition f32 scalars: bq columns for all experts + gcol
    nbq = sum(PDC[e] // P for e in range(E))
    scal = np.zeros((P, nbq + gcol.shape[1]), np.float32)
    boff = 0
    bk2 = None
    for e in range(E):
        hd, nhc = HD[e], NHC[e]
        heads = list(range(p * nhc, (p + 1) * nhc))
        scale = 1.0 / math.sqrt(hd) if ATYPE[e] in ("std", "loc") else 1.0
        d[f"wq{e}"] = _tile_w(_pad_cols(inputs[f"e{e}_Wq"], hd, heads) * scale)
        bqp = _pad_vec(inputs[f"e{e}_bq"], hd, heads) * scale
        scal[:, boff : boff + PDC[e] // P] = bqp.reshape(-1, P).T
        boff += PDC[e] // P
        wkp = _pad_cols(inputs[f"e{e}_Wk"], hd, heads)
        if e == 2:
            d["wk2"] = np.ascontiguousarray(wkp).astype(BF16)
            bk2 = _pad_vec(inputs["e2_bk"], hd, heads)
        else:
            d[f"wk{e}"] = _tile_w(wkp)
        wvp = np.zeros((H, PDV[e]), np.float32)
        wvp[:, : nhc * hd] = inputs[f"e{e}_Wv"][:, heads[0] * hd : (heads[-1] + 1) * hd]
        d[f"wv{e}"] = np.ascontiguousarray(wvp).astype(BF16)
        d[f"wo{e}"] = np.ascontiguousarray(
            _pad_rows(inputs[f"e{e}_Wo"], hd, heads)).astype(BF16)
    scal[:, nbq:] = gcol
    d["scal"] = scal
    d["bk2"] = np.ascontiguousarray(bk2[None, :]).astype(BF16)
    return d


# ---------------------------------------------------------------- device IR

@functools.lru_cache(maxsize=2)
def _build_nc(n0, n1, n2, bw):
    import concourse.mybir as mybir
    import concourse.tile as tile
    from concourse import bacc

    f32 = mybir.dt.float32
    bf16 = mybir.dt.bfloat16
    Exp = mybir.ActivationFunctionType.Exp
    Copy = mybir.ActivationFunctionType.Copy
    Ident = mybir.ActivationFunctionType.Identity

    NP = [n0, n1, n2, 4 * bw]
    QTILES = [[(q0, min(P, NP[e] - q0)) for q0 in range(0, NP[e], P)]
              for e in range(E)]
    GOFF = [0]
    for e in range(E):
        GOFF.append(GOFF[-1] + len(QTILES[e]))
    CWMAX = max(256, max(cw for e in range(3) for _, cw in _chunks(NP[e])))
    NBQ = sum(PDC[e] // P for e in range(E))
    BQOFF = [sum(PDC[i] // P for i in range(e)) for e in range(E)]
    NSCAL = NBQ + GOFF[-1]

    nc = bacc.Bacc(None, target_bir_lowering=False)

    x_t = nc.declare_dram_parameter("x_t", [H, S], bf16, isOutput=False)
    xg_d = [nc.declare_dram_parameter(f"xg{e}", [H, NP[e]], bf16, isOutput=False)
            for e in range(E)]
    scal_d = nc.declare_dram_parameter("scal", [P, NSCAL], f32, isOutput=False)
    maskc_d = nc.declare_dram_parameter("maskc", [P, E3MOFF[-1], bw], bf16,
                                        isOutput=False)
    bk2_d = nc.declare_dram_parameter("bk2", [1, PDC[2]], bf16, isOutput=False)
    wq_d, wk_d, wv_d, wo_d = [], [], [], []
    for e in range(E):
        wq_d.append(nc.declare_dram_parameter(
            f"wq{e}", [PDC[e] // P, P, HK * P], bf16, isOutput=False))
        if e == 2:
            wk_d.append(nc.declare_dram_parameter(
                "wk2", [H, PDC[2]], bf16, isOutput=False))
        else:
            wk_d.append(nc.declare_dram_parameter(
                f"wk{e}", [PDC[e] // P, P, HK * P], bf16, isOutput=False))
        wv_d.append(nc.declare_dram_parameter(f"wv{e}", [H, PDV[e]], bf16, isOutput=False))
        wo_d.append(nc.declare_dram_parameter(f"wo{e}", [PDC[e], H], bf16, isOutput=False))
    out_d = [nc.declare_dram_parameter(f"out{e}", [NP[e], H], bf16, isOutput=True)
             for e in range(E)]

    def pd_chunks(pdc):
        out, off = [], 0
        while off < pdc:
            w = min(512, pdc - off)
            out.append((off, w))
            off += w
        return out

    with tile.TileContext(nc) as tc:
        with (
            tc.tile_pool(name="singles", bufs=1) as singles,
            tc.tile_pool(name="xgpool", bufs=2) as xgpool,
            tc.tile_pool(name="ospool", bufs=2) as ospool,
            tc.tile_pool(name="wtpool", bufs=5) as wtpool,
            tc.tile_pool(name="wvpool", bufs=2) as wvpool,
            tc.tile_pool(name="wopool", bufs=2) as wopool,
            tc.tile_pool(name="apool", bufs=2) as apool,
            tc.tile_pool(name="otpool", bufs=2) as otpool,
            tc.tile_pool(name="epool", bufs=2) as epool,
            tc.tile_pool(name="tpool", bufs=2) as tpool,
            tc.tile_pool(name="psS", bufs=2, space="PSUM") as psS,
            tc.tile_pool(name="psA", bufs=2, space="PSUM") as psA,
            tc.tile_pool(name="psB", bufs=1, space="PSUM") as psB,
            tc.tile_pool(name="psC", bufs=1, space="PSUM") as psC,
        ):
            # ---- early loads: e3's compacted query gather (xg3) interleaved
            # with its wq ht-tiles so the first projection chain starts as
            # soon as the first half lands; xT (for K/V) follows.
            xT = singles.tile([P, HK, S], bf16)
            x_t_r = x_t.ap().rearrange("(o p) s -> p o s", p=P)

            def wt_dma(wdram, ht):
                wt = wtpool.tile([P, HK, P], bf16, tag="wt", name="wt")
                nc.sync.dma_start(
                    wt[:],
                    wdram.ap()[ht : ht + 1].rearrange(
                        "a p (o d) -> p (a o) d", d=P))
                return wt

            # Issue order: the first Q-proj chain needs only xg3's first
            # half + wq3 tile 0, so those two go absolutely first; scal
            # (bq/gcol, needed by the first QT bias add) rides right behind
            # them, well before the bulk xT transfers. wtpool has 5 slots,
            # so none of the 4 wq3 pre-DMAs blocks the in-order Sync queue
            # on slot reuse and wk3's first tile still finds a free slot.
            scal_sb = singles.tile([P, NSCAL], f32,
                                   padded_shape=[P, (NSCAL + 31) // 32 * 32])
            xg3_sb = singles.tile([P, HK, 4 * bw], bf16)
            xg3_r = xg_d[3].ap().rearrange("(o p) n -> p o n", p=P)
            wq3_pre = []
            nc.sync.dma_start(xg3_sb[:, 0:4], xg3_r[:, 0:4])
            wq3_pre.append(wt_dma(wq_d[3], 0))
            nc.sync.dma_start(scal_sb[:], scal_d.ap())
            nc.sync.dma_start(xg3_sb[:, 4:8], xg3_r[:, 4:8])
            for q in range(1, 4):
                wq3_pre.append(wt_dma(wq_d[3], q))
            for q in range(4):
                nc.sync.dma_start(xT[:, 2 * q : 2 * q + 2],
                                  x_t_r[:, 2 * q : 2 * q + 2])
            # NSCAL is not a multiple of 32 elems; pad the slot to keep every
            # subsequent SBUF tile 128B-aligned (PE streams ~20% slower on
            # misaligned operands).
            maskc_sb = singles.tile([P, E3MOFF[-1], bw], bf16)
            nc.sync.dma_start(maskc_sb[:], maskc_d.ap())
            bk2_sb = singles.tile([1, PDC[2]], bf16)
            nc.sync.dma_start(bk2_sb[:], bk2_d.ap())

            bq_sb = [scal_sb[:, BQOFF[e] : BQOFF[e] + PDC[e] // P]
                     for e in range(E)]
            gcol_sb = scal_sb[:, NBQ : NBQ + GOFF[-1]]

            def load_w(pool, dram, pdc, tag, trans=False, bufs=None):
                """[H, pdc] -> sbuf [P, HK, pdc]   (or [pdc, H] -> [P, pdc//P, H])"""
                if trans:
                    t = pool.tile([P, pdc // P, H], bf16, tag=tag,
                                  name=f"{tag}_t", bufs=bufs)
                    nc.sync.dma_start(t[:], dram.ap().rearrange("(o p) h -> p o h", p=P))
                else:
                    t = pool.tile([P, HK, pdc], bf16, tag=tag,
                                  name=f"{tag}_w", bufs=bufs)
                    r = dram.ap().rearrange("(o p) d -> p o d", p=P)
                    for half in range(2):
                        nc.sync.dma_start(t[:, 4 * half : 4 * half + 4],
                                          r[:, 4 * half : 4 * half + 4])
                return t

            ones_row = singles.tile([1, P], bf16)
            nc.vector.memset(ones_row[:], 1.0)
            ones_col = singles.tile([P, 1], bf16, padded_shape=[P, 64])
            nc.vector.memset(ones_col[:], 1.0)
            ones_mat = singles.tile([P, P], bf16)
            nc.vector.memset(ones_mat[:], 1.0)

            def proj_T(wdram, pdc, mv, nlen, pre=None):
                """QT/KT-style projection psums: [P(d-cols), chunk] = W.T @ mv.

                Stationary weight ht-tiles stream through wtpool (4-deep), so
                the Sync queue prefetches ahead while TensorE consumes."""
                for ht in range(pdc // P):
                    wt = (pre[ht] if pre and ht < len(pre)
                          else wt_dma(wdram, ht))
                    for (c0, cw) in _chunks(nlen):
                        ps = psA.tile([P, 512], f32, tag="mm", name="proj_ps")
                        for hk in range(HK):
                            nc.tensor.matmul(
                                ps[:, :cw],
                                wt[:, hk, :],
                                mv[:, hk, c0 : c0 + cw],
                                start=(hk == 0),
                                stop=(hk == HK - 1),
                            )
                        yield ps, ht, c0, cw

            def proj_nat(w_sb, pdc, bias_sb=None):
                """V-style natural projection psums: [P(s), chunk] = xT.T @ W."""
                for st in range(HK):
                    for (c0, cw) in pd_chunks(pdc):
                        ps = psA.tile([P, 512], f32, tag="mm", name="projn_ps")
                        for hk in range(HK):
                            nc.tensor.matmul(
                                ps[:, :cw],
                                xT[:, hk, st * P : (st + 1) * P],
                                w_sb[:, hk, c0 : c0 + cw],
                                start=(hk == 0),
                                stop=(hk == HK - 1 and bias_sb is None),
                            )
                        if bias_sb is not None:
                            nc.tensor.matmul(
                                ps[:, :cw],
                                ones_row[:, :P],
                                bias_sb[:, c0 : c0 + cw],
                                start=False, stop=True,
                            )
                        yield ps, st, c0, cw

            def elu_p1(ps, dst_ap, cw, bias=None):
                """dst = elu(ps + bias)+1 = exp(min(.,0)) + max(.,0), bf16."""
                tmin = tpool.tile([P, 512], bf16, tag="tmin", name="tmin", bufs=1)
                texp = tpool.tile([P, 512], bf16, tag="texp", name="texp", bufs=1)
                tmax = tpool.tile([P, 512], bf16, tag="tmax", name="tmax", bufs=1)
                if bias is None:
                    nc.vector.tensor_scalar_min(tmin[:, :cw], ps[:, :cw], 0.0)
                    nc.vector.tensor_scalar_max(tmax[:, :cw], ps[:, :cw], 0.0)
                else:
                    nc.vector.tensor_scalar(
                        tmin[:, :cw], ps[:, :cw], bias, 0.0,
                        mybir.AluOpType.add, mybir.AluOpType.min)
                    nc.vector.tensor_scalar(
                        tmax[:, :cw], ps[:, :cw], bias, 0.0,
                        mybir.AluOpType.add, mybir.AluOpType.max)
                nc.scalar.activation(texp[:, :cw], tmin[:, :cw], Exp)
                nc.vector.tensor_add(dst_ap, texp[:, :cw], tmax[:, :cw])

            def tree_sum(est, k0, nk, cw):
                """Sum est slots [P, k0:k0+nk, :cw] (bf16) -> [P, cw] AP.
                Wide 3D-AP adds keep the DVE instruction count at 2-3."""
                tr = tpool.tile([P, 4, CWMAX], bf16, tag="tr", name="tr", bufs=1)
                if nk == 8:
                    nc.vector.tensor_add(tr[:, 0:4, :cw], est[:, 0:4, :cw],
                                         est[:, 4:8, :cw])
                    nc.vector.tensor_add(tr[:, 0:2, :cw], tr[:, 0:2, :cw],
                                         tr[:, 2:4, :cw])
                elif nk == 4:
                    nc.vector.tensor_add(tr[:, 0:2, :cw],
                                         est[:, k0 : k0 + 2, :cw],
                                         est[:, k0 + 2 : k0 + 4, :cw])
                else:  # nk == 3
                    nc.vector.tensor_add(tr[:, 0, :cw], est[:, k0, :cw],
                                         est[:, k0 + 1, :cw])
                    nc.vector.tensor_add(tr[:, 0, :cw], tr[:, 0, :cw],
                                         est[:, k0 + 2, :cw])
                    return tr[:, 0, :cw]
                nc.vector.tensor_add(tr[:, 0, :cw], tr[:, 0, :cw], tr[:, 1, :cw])
                return tr[:, 0, :cw]

            def out_proj(e, ot, wo_sb, ti):
                q0, qp = QTILES[e][ti]
                gs = gcol_sb[:qp, GOFF[e] + ti : GOFF[e] + ti + 1]
                o_st = ospool.tile([P, 2, 512], bf16, tag="osb", name="o_st")
                for ci, (c0, cw) in enumerate(SCH):
                    ps = psA.tile([P, 512], f32, tag="mm", name="out_ps")
                    npt = PDC[e] // P
                    for pt in range(npt):
                        nc.tensor.matmul(
                            ps[:qp, :cw],
                            ot[:, pt, q0 : q0 + qp],
                            wo_sb[:, pt, c0 : c0 + cw],
                            start=(pt == 0), stop=(pt == npt - 1),
                        )
                    if ci == 0:
                        nc.scalar.activation(o_st[:qp, 0, :cw], ps[:qp, :cw],
                                             Copy, scale=gs)
                    else:
                        nc.vector.tensor_scalar_mul(o_st[:qp, 1, :cw],
                                                    ps[:qp, :cw], gs)
                # one merged [qp, 1024] bf16 store per q-tile, issued from the
                # Scalar HWDGE queue so weight prefetch on Sync never waits
                nc.scalar.dma_start(out_d[e].ap()[q0 : q0 + qp, :],
                                    o_st[:qp].rearrange("p a b -> p (a b)"))

            # ================= per-expert compute =================
            # Software-pipelined: expert e+1's projection matmul groups are
            # interleaved into expert e's attention loop, so TensorE has
            # independent work queued whenever attention stalls on ScalarE's
            # exp (the attention inner loop is exp-rate-bound).
            ctx = {}

            def proj_gen(e):
                """Emit expert e's projections / weight loads in steps."""
                pdc = PDC[e]
                pdv = PDV[e]
                nq = NP[e]
                if e != 3:
                    xg = xgpool.tile([P, HK, nq], bf16, tag="xg", name=f"xg{e}")
                    r = xg_d[e].ap().rearrange("(o p) n -> p o n", p=P)
                    for half in range(2):
                        nc.sync.dma_start(xg[:, 4 * half : 4 * half + 4],
                                          r[:, 4 * half : 4 * half + 4])
                else:
                    xg = xg3_sb

                if ATYPE[e] in ("std", "loc"):
                    QT = apool.tile([P, pdc // P, nq], bf16, tag="qt", name="QT")
                    for ps, ht, c0, cw in proj_T(wq_d[e], pdc, xg, nq,
                                                 pre=wq3_pre if e == 3 else None):
                        nc.vector.tensor_scalar_add(
                            QT[:, ht, c0 : c0 + cw], ps[:, :cw],
                            bq_sb[e][:, ht : ht + 1])
                        yield
                    KT = apool.tile([P, pdc // P, S], bf16, tag="kt", name="KT")
                    for ps, ht, c0, cw in proj_T(wk_d[e], pdc, xT, S):
                        nc.vector.tensor_copy(KT[:, ht, c0 : c0 + cw], ps[:, :cw])
                        yield
                else:  # linear: q' = elu(QT+bq)+1 ; k' natural = elu(K+bk)+1
                    QT = apool.tile([P, pdc // P, nq], bf16, tag="qt", name="QTl")
                    for ps, ht, c0, cw in proj_T(wq_d[e], pdc, xg, nq):
                        elu_p1(ps, QT[:, ht, c0 : c0 + cw], cw,
                               bias=bq_sb[e][:, ht : ht + 1])
                        yield
                    wk2_sb = load_w(wvpool, wk_d[2], PDC[2], "wk2", bufs=1)
                    KT = apool.tile([P, HK, pdc], bf16, tag="kt", name="Kn")
                    for ps, st, c0, cw in proj_nat(wk2_sb, pdc, bias_sb=bk2_sb):
                        elu_p1(ps, KT[:, st, c0 : c0 + cw], cw)
                        yield
                wv = load_w(wvpool, wv_d[e], pdv, "wv")
                V = apool.tile([P, HK, pdv], bf16, tag="v", name="V")
                for ps, st, c0, cw in proj_nat(wv, pdv):
                    nc.vector.tensor_copy(V[:, st, c0 : c0 + cw], ps[:, :cw])
                    yield
                ot = otpool.tile([P, pdc // P, nq], bf16, tag="ot", name=f"ot{e}")
                if HDV[e] < P:
                    # packed V: OT pad rows are never written; zero whole tile
                    nc.vector.memset(ot[:], 0.0)
                wo_sb = load_w(wopool, wo_d[e], pdc, "wo", trans=True)
                ctx[e] = (QT, KT, V, ot, wo_sb)

            def attn_gen(e):
                """Emit expert e's attention; yields after each matmul group."""
                pdc = PDC[e]
                hdv = HDV[e]
                nhc = NHC[e]
                nq = NP[e]
                QT, KT, V, ot, wo_sb = ctx[e]
                emitted = set()

                def emit_covered(upto):
                    for ti, (q0, qp) in enumerate(QTILES[e]):
                        if ti not in emitted and q0 + qp <= upto:
                            out_proj(e, ot, wo_sb, ti)
                            emitted.add(ti)

                if ATYPE[e] == "std":
                    for (c0, cw) in _chunks(nq):
                        for h in range(nhc):
                            est = epool.tile([P, HK, CWMAX], bf16, tag="est",
                                             name="est")
                            for kp in range(HK // 2):
                                st_ps = psS.tile([P, 2, 512], f32, tag="sc",
                                                 name="st_ps")
                                for sl in range(2):
                                    kt = 2 * kp + sl
                                    nc.tensor.matmul(
                                        st_ps[:, sl, :cw],
                                        KT[:, h, kt * P : (kt + 1) * P],
                                        QT[:, h, c0 : c0 + cw],
                                        start=True, stop=True,
                                    )
                                nc.scalar.activation(
                                    est[:, 2 * kp : 2 * kp + 2, :cw],
                                    st_ps[:, :, :cw], Exp)
                            o_ps = psB.tile([P, 512], f32, tag="ot", name="o_ps")
                            for kt in range(HK):
                                nc.tensor.matmul(
                                    o_ps[:hdv, :cw],
                                    V[:, kt, h * hdv : (h + 1) * hdv],
                                    est[:, kt, :cw],
                                    start=(kt == 0), stop=(kt == HK - 1),
                                )
                            es = tree_sum(est, 0, HK, cw)
                            den = psC.tile([P, 512], f32, tag="den", name="den")
                            nc.tensor.matmul(den[:hdv, :cw], ones_mat[:, :hdv],
                                             es, start=True, stop=True)
                            rcp = tpool.tile([P, 512], f32, tag="rcp", name="rcp")
                            nc.vector.reciprocal_approx_fast(
                                out=rcp[:hdv, :cw], in_=den[:hdv, :cw])
                            nc.vector.tensor_mul(
                                ot[:hdv, h, c0 : c0 + cw],
                                o_ps[:hdv, :cw], rcp[:hdv, :cw])
                            yield
                        emit_covered(c0 + cw)

                elif ATYPE[e] == "loc":
                    for qc in range(4):
                        kts = E3KTS[qc]
                        nm = len(kts)
                        for h in range(nhc):
                            est = epool.tile([P, 4, bw], bf16, tag="estl",
                                             name="estl")
                            for i0 in range(0, nm, 2):
                                grp = kts[i0 : i0 + 2]
                                st_ps = psS.tile([P, 2, 512], f32, tag="sc",
                                                 name="stl_ps")
                                for sl, kt in enumerate(grp):
                                    nc.tensor.matmul(
                                        st_ps[:, sl, :bw],
                                        KT[:, h, kt * P : (kt + 1) * P],
                                        QT[:, h, qc * bw : (qc + 1) * bw],
                                        start=True, stop=True,
                                    )
                                if len(grp) == 2:
                                    nc.scalar.activation(
                                        est[:, i0 : i0 + 2, :],
                                        st_ps[:, :, :bw], Exp)
                                else:
                                    nc.scalar.activation(
                                        est[:, i0, :], st_ps[:, 0, :bw], Exp)
                            nc.vector.tensor_mul(
                                est[:, 0:nm, :],
                                est[:, 0:nm, :],
                                maskc_sb[:, E3MOFF[qc] : E3MOFF[qc] + nm, :])
                            o_ps = psB.tile([P, 512], f32, tag="ot", name="ol_ps")
                            for i, kt in enumerate(kts):
                                nc.tensor.matmul(
                                    o_ps[:, :bw],
                                    V[:, kt, h * P : (h + 1) * P],
                                    est[:, i, :],
                                    start=(i == 0), stop=(i == nm - 1),
                                )
                            es = tree_sum(est, 0, nm, bw)
                            den = psC.tile([P, 512], f32, tag="den", name="denl")
                            nc.tensor.matmul(den[:, :bw], ones_mat[:],
                                             es, start=True, stop=True)
                            rcp = tpool.tile([P, 512], f32, tag="rcp", name="rcpl")
                            nc.vector.reciprocal_approx_fast(
                                out=rcp[:, :bw], in_=den[:, :bw])
                            nc.vector.tensor_mul(
                                ot[:, h, qc * bw : (qc + 1) * bw],
                                o_ps[:, :bw], rcp[:, :bw])
                            yield
                        emit_covered((qc + 1) * bw)

                else:  # linear
                    nhc_ = nhc
                    kv_sb = tpool.tile([P, 4, P], bf16, tag="kv_sb", name="kv_sb", bufs=1)
                    ks_bc = tpool.tile([P, 4, P], bf16, tag="ks_bc", name="ks_bc", bufs=1)
                    # ksum row [1, pdv] = sum_s k' for all heads in one sweep
                    pdv = PDV[e]
                    ksr_ps = psC.tile([P, 512], f32, tag="den", name="ksr_ps")
                    for st in range(HK):
                        nc.tensor.matmul(
                            ksr_ps[:1, :pdv],
                            ones_col[:, :],
                            KT[:, st, :pdv],
                            start=(st == 0), stop=(st == HK - 1),
                        )
                    ksr_sb = tpool.tile([1, 512], bf16, tag="ksr", name="ksr_sb", bufs=1)
                    nc.scalar.activation(ksr_sb[:, :pdv], ksr_ps[:1, :pdv], Copy)
                    yield
                    for h in range(nhc_):
                        kv_ps = psB.tile([P, 512], f32, tag="ot", name="kv_ps")
                        for st in range(HK):
                            nc.tensor.matmul(
                                kv_ps[:, :P],
                                KT[:, st, h * P : (h + 1) * P],
                                V[:, st, h * P : (h + 1) * P],
                                start=(st == 0), stop=(st == HK - 1),
                            )
                        nc.scalar.activation(kv_sb[:, h, :], kv_ps[:, :P], Copy)
                        # broadcast ksum row across partitions -> [P, P] lhsT
                        kb_ps = psC.tile([P, 512], f32, tag="den", name="kb_ps")
                        nc.tensor.matmul(
                            kb_ps[:, :P],
                            ksr_sb[:, h * P : (h + 1) * P],
                            ones_row[:, :P],
                            start=True, stop=True,
                        )
                        nc.scalar.activation(ks_bc[:, h, :], kb_ps[:, :P], Copy)
                        yield
                    for (c0, cw) in _chunks(nq):
                        for h in range(nhc_):
                            num_ps = psA.tile([P, 512], f32, tag="mm",
                                              name="num_ps")
                            nc.tensor.matmul(
                                num_ps[:, :cw],
                                kv_sb[:, h, :],
                                QT[:, h, c0 : c0 + cw],
                                start=True, stop=True,
                            )
                            den = psC.tile([P, 512], f32, tag="den", name="den2")
                            nc.tensor.matmul(
                                den[:, :cw],
                                ks_bc[:, h, :],
                                QT[:, h, c0 : c0 + cw],
                                start=True, stop=True,
                            )
                            rcp = tpool.tile([P, 512], f32, tag="rcp", name="rcp2")
                            nc.vector.reciprocal_approx_fast(
                                out=rcp[:, :cw], in_=den[:, :cw])
                            nc.vector.tensor_mul(
                                ot[:, h, c0 : c0 + cw],
                                num_ps[:, :cw], rcp[:, :cw])
                            yield
                        emit_covered(c0 + cw)

            ORDER = (3, 0, 1, 2)
            for _ in proj_gen(3):
                pass
            for idx, e in enumerate(ORDER):
                nxt = ORDER[idx + 1] if idx + 1 < len(ORDER) else None
                pg = proj_gen(nxt) if nxt is not None else None
                for gi, _ in enumerate(attn_gen(e)):
                    if pg is not None:
                        for _ in range(2):
                            if next(pg, "done") == "done":
                                pg = None
                                break
                if pg is not None:
                    for _ in pg:
                        pass

    nc.finalize()
    return nc


# ---------------------------------------------------------------- entry

def kernel(**inputs) -> np.ndarray:
    from concourse.bass_utils import run_bass_kernel_spmd

    inputs = {k: np.asarray(v, np.float32) if np.asarray(v).dtype.kind == "f"
              else np.asarray(v) for k, v in inputs.items()}
    gatesT = [_host_gates(inputs["x"][b], inputs["Wg"]) for b in range(4)]
    bo_eff_all = np.stack([
        inputs[f"e{e}_bv"] @ inputs[f"e{e}_Wo"] + inputs[f"e{e}_bo"]
        for e in range(E)
    ])
    # top-2 selection indices per (batch, expert); shared padding across batches
    sel = [[np.nonzero(gatesT[b][e] > 0)[0] for e in range(3)] for b in range(4)]
    # 64-granular padding keeps every per-head free-dim stride a multiple of
    # 128 bytes (SBUF-aligned PE streaming)
    npad = [max(P, -(-max(len(sel[b][e]) for b in range(4)) // 64) * 64)
            for e in range(3)]
    # e3 (local attention): block-compacted queries, one pad width per
    # 256-token block across all batches
    pos3 = [[np.nonzero(gatesT[b][3][qc * 256 : (qc + 1) * 256] > 0)[0]
             + qc * 256 for qc in range(4)] for b in range(4)]
    bw = max(64, -(-max(len(pos3[b][qc]) for b in range(4) for qc in range(4))
                   // 64) * 64)
    in_maps = [
        _prep_core(inputs, c // 2, c % 2, gatesT[c // 2],
                   sel[c // 2], pos3[c // 2], npad, bw)
        for c in range(N_CORES)
    ]
    nc = _build_nc(npad[0], npad[1], npad[2], bw)
    trace = bool(int(os.environ.get("KERNEL_TRACE", "0")))
    if trace:
        import jax

        jax.devices()  # force axon platform registration
        try:
            from antenv.axon_hooks import (
                get_axon_ntff_profile_hook,
                set_axon_ntff_profile_hook,
            )

            if get_axon_ntff_profile_hook() is None:
                from trn_agent_boot.trn_boot import _ntff_profile_via_ctypes

                set_axon_ntff_profile_hook(
                    _ntff_profile_via_ctypes("/opt/axon/libaxon_pjrt.so"))
        except Exception as exc:  # tracing is best-effort
            print(f"NTFF hook setup failed: {exc}")
    res = run_bass_kernel_spmd(nc, in_maps, list(range(N_CORES)), trace=trace)
    if trace and res.exec_time_ns is not None:
        print(f"HW exec time: {res.exec_time_ns} ns")
    out = np.empty((4, S, H), np.float32)
    for b in range(4):
        acc = gatesT[b].T @ bo_eff_all  # gated output-bias term, host-side
        for p in range(2):
            r = res.results[2 * b + p]
            o3 = np.asarray(r["out3"], np.float32)
            for qc in range(4):
                idx = pos3[b][qc]
                acc[idx] += o3[qc * bw : qc * bw + len(idx)]
            for e in range(3):
                idx = sel[b][e]
                acc[idx] += np.asarray(r[f"out{e}"][: len(idx)], np.float32)
        out[b] = acc
    return out

